# revision 4
# baseline (speedup 1.0000x reference)
"""Trainium2 Bass kernel for nn_Conv2d_NN (retrieval-knn conv).

Math: x -> concat coords -> pixel_unshuffle(2) -> tokens x2 [136, 1024] per batch;
dist = all-pairs sq-euclidean over tokens; idx = top-9 nearest (incl self);
y = conv1d over gathered neighbors; pixel_shuffle; pointwise conv.

Strategy (8 cores, data-parallel over batch, 4 batches/core):
- Host folds pixel_shuffle+pointwise into the conv weights: V_k = fold(pw_w, w1[:,:,k])
  giving 9 matrices [136 -> 128 outputs]; bias folded in via an extra ones-row.
- Device per batch: Gv_k = V_k @ x2 (fp32r matmuls, stacked [128, 9216]);
  ranking r[n,m] = dot(x2_n, x2_m) - 0.5*sq[m] via fp32 matmuls with an extended
  contraction row; self excluded by adding -1e30 on the diagonal; top-8 neighbors
  per row with DVE max/max_index; combined indices round-trip through DRAM into
  the gpsimd ap_gather wrapped layout; gather + reduce over the 8 neighbor maps
  + self map (k=0, bias folded) -> final [128, 1024] -> host reshapes.
Self is always the nearest neighbor (dist ~0 vs >>0 for others), so top-8 of the
diag-masked ranking == reference idx[:, 1:9]; reference idx[:, 0] == self.
"""
from contextlib import ExitStack

import numpy as np

import concourse.bacc as bacc
import concourse.mybir as mybir
import concourse.tile as tile
import concourse.bass_utils as bu
from concourse import library_config

B, CIN, H, W = 32, 32, 64, 64
S, K = 2, 9
C1 = (CIN + 2) * S * S          # 136
N = (H // S) * (W // S)         # 1024
NCORES = 8
BPC = B // NCORES               # batches per core
P = 128
NT = N // P                     # 8 n-tiles per batch
NB = N // 512                   # 2 moving-dim blocks

F32 = mybir.dt.float32
F32R = mybir.dt.float32r
U16 = mybir.dt.uint16
I16 = mybir.dt.int16


def _features(x: np.ndarray) -> np.ndarray:
    """[B, 32, 64, 64] -> [B, 136, 1024] float32 (coords + unshuffle + flatten)."""
    b = x.shape[0]
    xg, yg = np.meshgrid(np.arange(H, dtype=np.float32),
                         np.arange(W, dtype=np.float32), indexing="ij")
    nrm = np.sqrt(xg * xg + yg * yg).astype(np.float32)
    nrm = np.maximum(nrm, np.float32(1e-12))
    cx = (xg / nrm).astype(np.float32)
    cy = (yg / nrm).astype(np.float32)
    coords = np.broadcast_to(np.stack([cx, cy])[None], (b, 2, H, W))
    xc = np.concatenate([x, coords], axis=1)                      # [b, 34, H, W]
    u = xc.reshape(b, CIN + 2, H // S, S, W // S, S)
    u = u.transpose(0, 1, 3, 5, 2, 4).reshape(b, C1, N)           # [b, 136, 1024]
    return np.ascontiguousarray(u.astype(np.float32))


def _build_device_inputs(x, w1, b1, pw_w, pw_b):
    x2 = _features(np.asarray(x, dtype=np.float32))               # [B, 136, 1024]
    sq = np.einsum("bcn,bcn->bn", x2, x2, dtype=np.float32,
                   casting="same_kind").astype(np.float32)        # [B, 1024]

    mains = np.ascontiguousarray(x2[:, :P, :])                    # [B, 128, 1024]
    tailL = np.zeros((B, 16, N), dtype=np.float32)
    tailR = np.zeros((B, 16, N), dtype=np.float32)
    tailL[:, 0:8] = x2[:, 128:136]
    tailL[:, 8] = 1.0
    tailR[:, 0:8] = x2[:, 128:136]
    tailR[:, 8] = -0.5 * sq
    tailR[:, 9] = 1.0

    # Fold pixel_shuffle + pointwise conv into per-k weight mats V_k [128, 136].
    w1r = np.asarray(w1, dtype=np.float64).reshape(CIN + 2, S * S, C1, K)
    V = np.einsum("ob,bqck->oqck", np.asarray(pw_w, dtype=np.float64), w1r)
    V = V.reshape(P, C1, K)                                       # [128, 136, 9]
    bfold = (np.asarray(pw_w, np.float64) @ np.asarray(b1, np.float64)
             .reshape(CIN + 2, S * S).sum(axis=1) if False else
             np.einsum("ob,bq->oq", np.asarray(pw_w, np.float64),
                       np.asarray(b1, np.float64).reshape(CIN + 2, S * S)))
    # bias: out[o', n] += sum_c34 pw_w[o2,c34] b1[4c34+q] + pw_b[o2]
    b_out = (bfold.reshape(P) + np.repeat(np.asarray(pw_b, np.float64), S * S))
    # laid out [rows, k*128 + col] so the SBUF load is a plain 2D copy
    vt_main = np.zeros((P, K * P), dtype=np.float32)              # rows 0..127 of V_k^T
    vt_tail = np.zeros((16, K * P), dtype=np.float32)             # rows 128..143
    for k in range(K):
        vt_main[:, k * P:(k + 1) * P] = V[:, :P, k].T.astype(np.float32)
        vt_tail[0:8, k * P:(k + 1) * P] = V[:, 128:136, k].T.astype(np.float32)
    vt_tail[9, 0:P] = b_out.astype(np.float32)                    # pairs ones-row (k=0)

    diag = np.zeros((P, P), dtype=np.float32)
    np.fill_diagonal(diag, np.float32(-1e30))
    kofs = np.broadcast_to(
        (np.arange(1, 9, dtype=np.uint16) * np.uint16(1024))[None, :], (P, 8)
    ).copy()

    shared = dict(vt_main=vt_main, vt_tail=vt_tail, diag=diag, kofs=kofs)
    per_core = []
    for c in range(NCORES):
        sl = slice(c * BPC, (c + 1) * BPC)
        per_core.append(dict(
            mains=np.ascontiguousarray(mains[sl]),
            tailL=np.ascontiguousarray(tailL[sl]),
            tailR=np.ascontiguousarray(tailR[sl]),
            **shared,
        ))
    return per_core


def _build_nc():
    nc = bacc.Bacc("TRN2", target_bir_lowering=False, debug=False,
                   num_devices=NCORES)
    mains_d = nc.dram_tensor("mains", [BPC, P, N], F32, kind="ExternalInput")
    tailL_d = nc.dram_tensor("tailL", [BPC, 16, N], F32, kind="ExternalInput")
    tailR_d = nc.dram_tensor("tailR", [BPC, 16, N], F32, kind="ExternalInput")
    vtm_d = nc.dram_tensor("vt_main", [P, K * P], F32, kind="ExternalInput")
    vtt_d = nc.dram_tensor("vt_tail", [16, K * P], F32, kind="ExternalInput")
    diag_d = nc.dram_tensor("diag", [P, P], F32, kind="ExternalInput")
    kofs_d = nc.dram_tensor("kofs", [P, 8], U16, kind="ExternalInput")
    out_d = nc.dram_tensor("out", [BPC, P, N], F32, kind="ExternalOutput")

    with tile.TileContext(nc) as tc:
        with ExitStack() as ctx:
            consts = ctx.enter_context(tc.tile_pool(name="consts", bufs=1))
            feats = ctx.enter_context(tc.tile_pool(name="feats", bufs=2))
            big = ctx.enter_context(tc.tile_pool(name="big", bufs=1))
            small = ctx.enter_context(tc.tile_pool(name="small", bufs=2))
            idxp = ctx.enter_context(tc.tile_pool(name="idxp", bufs=2))
            dram = ctx.enter_context(tc.tile_pool(name="dram", bufs=2, space="DRAM"))
            psg = ctx.enter_context(tc.tile_pool(name="psg", bufs=2, space="PSUM"))
            psr = ctx.enter_context(tc.tile_pool(name="psr", bufs=2, space="PSUM"))

            nc.gpsimd.load_library(library_config.ap_gather)

            # constants
            vtm = consts.tile([P, K * P], F32)       # vt_main[k] at cols k*128
            nc.sync.dma_start(vtm[:], vtm_d.ap())
            vtt = consts.tile([16, K * P], F32)
            nc.sync.dma_start(vtt[:], vtt_d.ap())
            vtmr = consts.tile([P, K * P], F32R)     # rounded copies for fp32r mm
            nc.any.tensor_copy(vtmr[:], vtm[:])
            vttr = consts.tile([16, K * P], F32R)
            nc.any.tensor_copy(vttr[:], vtt[:])
            diag = consts.tile([P, P], F32)
            nc.sync.dma_start(diag[:], diag_d.ap())
            kofs = consts.tile([P, 8], U16)
            nc.sync.dma_start(kofs[:], kofs_d.ap())

            for b in range(BPC):
                main = feats.tile([P, N], F32, tag="main")
                tl = feats.tile([16, N], F32, tag="tl")
                tr = feats.tile([16, N], F32, tag="tr")
                nc.sync.dma_start(main[:], mains_d.ap()[b])
                nc.sync.dma_start(tl[:], tailL_d.ap()[b])
                nc.sync.dma_start(tr[:], tailR_d.ap()[b])
                mainr_t = feats.tile([P, N], F32R, tag="mainr")
                nc.any.tensor_copy(mainr_t[:], main[:])
                trr_t = feats.tile([16, N], F32R, tag="trr")
                nc.any.tensor_copy(trr_t[:], tr[:])
                mainr = mainr_t[:]
                trr = trr_t[:]

                # ---- Gv_k = V_k @ x2 (+bias via ones row), fp32r ----
                gvcat = big.tile([P, K * N], F32, tag="gvcat")
                for k in range(K):
                    gps = psg.tile([P, N], F32, tag="gv")
                    for nb in range(NB):
                        cs = slice(nb * 512, (nb + 1) * 512)
                        nc.tensor.matmul(gps[:, cs],
                                         vtmr[:, k * P:(k + 1) * P],
                                         mainr[:, cs], start=True, stop=False)
                        nc.tensor.matmul(gps[:, cs],
                                         vttr[0:10, k * P:(k + 1) * P],
                                         trr[0:10, cs], start=False, stop=True)
                    nc.any.tensor_copy(gvcat[:, k * N:(k + 1) * N], gps[:])

                # ---- ranking r + top8 per n-tile ----
                idx_dram = dram.tile([N, 8], U16, tag="idxd")
                for nt in range(NT):
                    ms = slice(nt * P, (nt + 1) * P)
                    rps = psr.tile([P, N], F32, tag="r")
                    for nb in range(NB):
                        cs = slice(nb * 512, (nb + 1) * 512)
                        nc.tensor.matmul(rps[:, cs], main[:, ms], main[:, cs],
                                         start=True, stop=False)
                        nc.tensor.matmul(rps[:, cs], tl[0:10, ms], tr[0:10, cs],
                                         start=False, stop=True)
                    # mask self on the diagonal block
                    nc.vector.tensor_add(rps[:, ms], rps[:, ms], diag[:])
                    mx = small.tile([P, 8], F32, tag="mx")
                    mi = small.tile([P, 8], U16, tag="mi")
                    nc.vector.max(out=mx[:], in_=rps[:])
                    nc.vector.max_index(out=mi[:], in_max=mx[:], in_values=rps[:])
                    nc.vector.tensor_add(mi[:], mi[:], kofs[:])
                    nc.sync.dma_start(idx_dram[ms, :], mi[:])

                # ---- wrap indices into ap_gather layout ----
                # flat i = n*8 + j -> [i%16, i//16]; value = (j+1)*1024 + mi[n, j]
                wrap = idxp.tile([P, 512], U16, tag="wrap")
                src = idx_dram[:].rearrange("(f h) j -> h j f", h=2)  # [2, 8, 512]
                nc.sync.dma_start(wrap[0:16, :], src)
                for g in range(1, 8):
                    nc.sync.dma_start(wrap[g * 16:(g + 1) * 16, :], wrap[0:16, :])

                # ---- gather + reduce ----
                gout = big.tile([P, 8 * N], F32, tag="gout")
                nc.gpsimd.ap_gather(gout[:], gvcat[:], wrap[:].bitcast(I16),
                                    channels=P, num_elems=K * N, d=1,
                                    num_idxs=8 * N)
                fin = small.tile([P, N], F32, tag="fin")
                nc.vector.tensor_reduce(
                    fin[:], gout[:].rearrange("p (n j) -> p n j", j=8),
                    axis=mybir.AxisListType.X, op=mybir.AluOpType.add)
                nc.vector.tensor_add(fin[:], fin[:], gvcat[:, 0:N])
                nc.sync.dma_start(out_d.ap()[b], fin[:])

    nc.finalize()
    return nc


_NC_CACHE = {}


def kernel(x, w1, b1, pw_w, pw_b):
    per_core = _build_device_inputs(x, w1, b1, pw_w, pw_b)
    if "nc" not in _NC_CACHE:
        _NC_CACHE["nc"] = _build_nc()
    nc = _NC_CACHE["nc"]
    res = bu.run_bass_kernel_spmd(nc, per_core, core_ids=list(range(NCORES)))
    outs = np.concatenate([r["out"] for r in res.results], axis=0)  # [B, 128, 1024]
    f = outs.reshape(B, CIN, S, S, H // S, W // S)
    out = f.transpose(0, 1, 4, 2, 5, 3).reshape(B, CIN, H, W)
    return np.ascontiguousarray(out.astype(np.float32))
